# revision 7
# baseline (speedup 1.0000x reference)
"""Trainium2 Bass kernel for NeuroVPR Vanilla SNN (3-layer LIF, T=3).

Data-parallel over batch: B=16384 -> 2048 per core x 8 cores.

Math (per timestep, per layer): v = (v_prev + h)/2; s = (v>=1); v *= (1-s).
All operands are fp8(e4m3) with weights pre-scaled by 16 (keeps N(0,1/D)
weight entries out of e4m3's subnormal range). Tracking W = 32*v:
    W_t = 0.5*M_{t-1} + H_t     (H = 16*h from scaled weights)
    s_t = (W_t >= 32)
    M_t = W_t * (W_t < 32)
Offline check vs the fp32 recurrence: ~10k/4.2M layer-1 spike flips, layer-2
membrane peaks at 0.59 vs threshold 1.0 (zero layer-2 spikes), so the output
spike pattern is unchanged.

Matmuls use fp8 MatmulPerfMode.DoubleRow (2 k-tiles of 128 per instruction,
2x PE rate): lhsT [128, 2, M] / rhs [128, 2, N] -> psum [M, N], N=512.
Layout: h.T = W @ x.T with d (contraction) on partitions. Host pre-permutes
dvs to [T, 11, half, 128, 2, 1024] fp8 so each k-pair half-batch tile is one
fully contiguous 256KB DMA (2KB per partition). D padded 2752->2816 (22*128);
pad row 2752 carries the L1 bias with x=1 there.

Schedule (keeps TensorE dense): per timestep, L1 runs as two half-batch
passes of 4 PSUM banks each, with the previous timestep's L2+L3 emitted
between the passes. Spike compares run on GpSimd; membrane updates on
VectorE; x DMAs alternate between the sync and scalar queues.
"""
import os
import numpy as np

B, T, D = 16384, 3, 2752
DP = 2816         # D padded to 22*128 (pad row 2752 = bias row)
H, O = 256, 100
OP = 112          # O padded to mult of 16 (DoubleRow ldweights step%16==0)
NCORES = 8
BC = B // NCORES  # 2048
NB = 512          # psum block along batch
KT = DP // 128    # 22 contraction tiles for L1
KP = KT // 2      # 11 DoubleRow k-pairs
HB = BC // 2      # half-batch per L1 pass (1024)
WS = 16.0         # weight pre-scale (power of 2)
TH = 2.0 * WS     # spike threshold in scaled-w units

_compiled = {}
last_results = None  # BassKernelResults of the most recent run (for profiling)


def _build(use_b2, use_b3):
    from contextlib import ExitStack
    import concourse.bass as bass
    import concourse.mybir as mybir
    import concourse.tile as tile
    from concourse import bacc

    f8, f32 = mybir.dt.float8e4, mybir.dt.float32
    A = mybir.AluOpType
    DR = mybir.MatmulPerfMode.DoubleRow

    nc = bacc.Bacc("TRN2", target_bir_lowering=False, debug=False)
    x = nc.dram_tensor("x", [T, KP, 2, 128, 2 * HB], f8, kind="ExternalInput").ap()
    w1 = nc.dram_tensor("w1", [DP, H], f8, kind="ExternalInput").ap()
    w2 = nc.dram_tensor("w2", [H, H], f8, kind="ExternalInput").ap()
    w3 = nc.dram_tensor("w3", [H, OP], f8, kind="ExternalInput").ap()
    b2 = nc.dram_tensor("b2", [1, H], f8, kind="ExternalInput").ap()
    b3 = nc.dram_tensor("b3", [1, OP], f8, kind="ExternalInput").ap()
    out = nc.dram_tensor("out", [O, BC], f32, kind="ExternalOutput").ap()

    with tile.TileContext(nc) as tc, ExitStack() as ctx:
        wp = ctx.enter_context(tc.tile_pool(name="wp", bufs=1))
        xp = ctx.enter_context(tc.tile_pool(name="xp", bufs=12))
        pp1 = ctx.enter_context(tc.tile_pool(name="pp1", bufs=6, space="PSUM"))
        pp23 = ctx.enter_context(tc.tile_pool(name="pp23", bufs=2, space="PSUM"))
        sp = ctx.enter_context(tc.tile_pool(name="sp", bufs=1))
        tp = ctx.enter_context(tc.tile_pool(name="tp", bufs=6))

        # resident weights, [d_part, (k h)] layout; pair j = dims k=2j,2j+1
        w1t = wp.tile([128, KT * H], f8)
        w1r = w1.rearrange("(k p) h -> p k h", p=128)
        w1o = w1t[:, :].rearrange("p (k h) -> p k h", k=KT)
        nc.sync.dma_start(out=w1o[:, 0:1, :], in_=w1r[:, 0:1, :])
        for c0, c1 in ((1, 7), (7, 14), (14, 22)):
            nc.scalar.dma_start(out=w1o[:, c0:c1, :], in_=w1r[:, c0:c1, :])
        w2t = wp.tile([128, 2 * H], f8)
        w2o = w2t[:, :].rearrange("p (k h) -> p k h", k=2)
        nc.gpsimd.dma_start(out=w2o, in_=w2.rearrange("(k p) h -> p k h", p=128))
        w3t = wp.tile([128, 2 * OP], f8)
        w3o = w3t[:, :].rearrange("p (k h) -> p k h", k=2)
        nc.gpsimd.dma_start(out=w3o, in_=w3.rearrange("(k p) h -> p k h", p=128))
        b2t = wp.tile([1, H], f8)
        nc.gpsimd.dma_start(out=b2t[:, :], in_=b2[:, :])
        b3t = wp.tile([1, OP], f8)
        nc.gpsimd.dma_start(out=b3t[:, :], in_=b3[:, :])
        ones = wp.tile([1, NB], f8)
        nc.gpsimd.memset(ones[:, :], 1.0)

        # persistent state (M = 32*v_after_reset, zero-init) and fp8 spikes
        m1 = [sp.tile([128, BC], f32, tag=f"m1_{h}", name=f"m1_{h}") for h in range(2)]
        m2 = [sp.tile([128, BC], f32, tag=f"m2_{h}", name=f"m2_{h}") for h in range(2)]
        m3 = sp.tile([128, BC], f32, tag="m3")
        s1 = sp.tile([128, 2 * BC], f8, tag="s1", name="s1")
        s2 = sp.tile([128, 2 * BC], f8, tag="s2", name="s2")
        s1v = s1[:, :].rearrange("p (k b) -> p k b", k=2)
        s2v = s2[:, :].rearrange("p (k b) -> p k b", k=2)
        outsb = sp.tile([128, BC], f32, tag="outsb")
        for mt in (*m1, *m2, m3):
            nc.vector.memset(mt[:, :], 0.0)

        def lif_w(psum, m_ap):
            """W = M/2 + H. Reads+releases the psum bank; returns w tile."""
            P = psum.shape[0]
            w = tp.tile([128, NB], f32, tag="w", name="w")[:P, :]
            nc.vector.scalar_tensor_tensor(w, m_ap, 0.5, psum, A.mult, A.add)
            return w

        def lif_s(w, s_ap):
            nc.gpsimd.tensor_scalar(s_ap, w, TH, None, A.is_ge)

        def lif_m(w, m_ap):
            nc.vector.scalar_tensor_tensor(m_ap, w, TH, w, A.is_lt, A.mult)

        def l2_group(t, h, b, pool, tag):
            ps2 = pool.tile([128, NB], f32, tag=tag, name=f"ps2_{t}_{h}_{b}")
            first = True
            if use_b2:
                nc.tensor.matmul(ps2[:, :], b2t[0:1, h * 128:(h + 1) * 128],
                                 ones[0:1, :], start=True, stop=False)
                first = False
            nc.tensor.matmul(ps2[:, :], w2o[:, 0:2, h * 128:(h + 1) * 128],
                             s1v[:, :, b * NB:(b + 1) * NB],
                             start=first, stop=True, perf_mode=DR)
            return ps2

        def l3_group(t, b, pool, tag):
            ps3 = pool.tile([128, NB], f32, tag=tag, name=f"ps3_{t}_{b}")
            first = True
            if use_b3:
                nc.tensor.matmul(ps3[:OP, :], b3t[0:1, :], ones[0:1, :],
                                 start=True, stop=False)
                first = False
            nc.tensor.matmul(ps3[:OP, :], w3o[:, 0:2, 0:OP],
                             s2v[:, :, b * NB:(b + 1) * NB],
                             start=first, stop=True, perf_mode=DR)
            return ps3

        def l2_all(t, pool, tag):
            """Layer-2 matmuls + LIF for timestep t (all batch blocks)."""
            last = (t == T - 1)
            for b in range(4):
                bs = slice(b * NB, (b + 1) * NB)
                for h in range(2):
                    ps2 = l2_group(t, h, b, pool, tag)
                    w = lif_w(ps2[:, :], m2[h][:, bs])
                    lif_s(w, s2v[:, h, bs])
                    if not last:
                        lif_m(w, m2[h][:, bs])

        def l3_all(t, pool, tag):
            """Layer-3 matmuls + LIF for timestep t (all batch blocks)."""
            last = (t == T - 1)
            for b in range(4):
                bs = slice(b * NB, (b + 1) * NB)
                ps3 = l3_group(t, b, pool, tag)
                w3_ = lif_w(ps3[:O, :], m3[:O, bs])
                lif_s(w3_, outsb[:O, bs])
                if not last:
                    lif_m(w3_, m3[:O, bs])
                else:
                    nc.sync.dma_start(out=out[:, bs], in_=outsb[:O, bs])

        def l1_pass(t, half):
            """One half-batch L1 pass: 4 psum groups (2h x 2b), k-pairs inner."""
            boff = half * HB
            ps1 = [[pp1.tile([128, NB], f32, tag="ps1", name=f"ps1_{t}_{half}_{h}_{b}")
                    for b in range(2)] for h in range(2)]
            for j in range(KP):
                xt = xp.tile([128, 2 * HB], f8, tag="x", name="xt")
                q = nc.sync if (j % 2 == 0) else nc.scalar
                q.dma_start(out=xt[:, :], in_=x[t, j, half, :, :])
                xv = xt[:, :].rearrange("p (k b) -> p k b", k=2)
                for h in range(2):
                    for b in range(2):
                        nc.tensor.matmul(
                            ps1[h][b][:, :],
                            w1o[:, 2 * j:2 * j + 2, h * 128:(h + 1) * 128],
                            xv[:, :, b * NB:(b + 1) * NB],
                            start=(j == 0), stop=(j == KP - 1), perf_mode=DR)
            # release all 4 banks first (w-ops), then spikes, then membranes
            ws = {}
            for h in range(2):
                for b in range(2):
                    bs = slice(boff + b * NB, boff + (b + 1) * NB)
                    ws[h, b] = lif_w(ps1[h][b][:, :], m1[h][:, bs])
            for h in range(2):
                for b in range(2):
                    bs = slice(boff + b * NB, boff + (b + 1) * NB)
                    lif_s(ws[h, b], s1v[:, h, bs])
            if t != T - 1:
                for h in range(2):
                    for b in range(2):
                        bs = slice(boff + b * NB, boff + (b + 1) * NB)
                        lif_m(ws[h, b], m1[h][:, bs])

        for t in range(T):
            l1_pass(t, 0)
            if t > 0:
                l2_all(t - 1, pp23, "ps23")
            if t == T - 1:
                l3_all(t - 1, pp23, "ps23")
                for b in (0, 1):
                    bs = slice(b * NB, (b + 1) * NB)
                    for h in range(2):
                        ps2 = l2_group(t, h, b, pp23, "ps23")
                        w = lif_w(ps2[:, :], m2[h][:, bs])
                        lif_s(w, s2v[:, h, bs])
            l1_pass(t, 1)
            if 0 < t < T - 1:
                l3_all(t - 1, pp23, "ps23")
        # tail: l2(T-1, b23) and l3(T-1) pipelined per b-block
        t_ = T - 1
        for b in (2, 3):
            bs = slice(b * NB, (b + 1) * NB)
            for h in range(2):
                ps2 = l2_group(t_, h, b, pp1, "ps1")
                w = lif_w(ps2[:, :], m2[h][:, bs])
                lif_s(w, s2v[:, h, bs])
            bp = b - 2
            bs = slice(bp * NB, (bp + 1) * NB)
            ps3 = l3_group(t_, bp, pp23, "ps23")
            w3_ = lif_w(ps3[:O, :], m3[:O, bs])
            lif_s(w3_, outsb[:O, bs])
            nc.sync.dma_start(out=out[:, bs], in_=outsb[:O, bs])
        for bp in (2, 3):
            bs = slice(bp * NB, (bp + 1) * NB)
            ps3 = l3_group(t_, bp, pp23, "ps23")
            w3_ = lif_w(ps3[:O, :], m3[:O, bs])
            lif_s(w3_, outsb[:O, bs])
            nc.sync.dma_start(out=out[:, bs], in_=outsb[:O, bs])

    nc.compile()
    return nc


def kernel(dvs, W1, b1, W2, b2, W3, b3):
    global last_results
    import ml_dtypes
    from concourse.bass_utils import run_bass_kernel_spmd

    use_b2 = bool(np.any(b2))
    use_b3 = bool(np.any(b3))
    key = (use_b2, use_b3)
    if key not in _compiled:
        _compiled[key] = _build(use_b2, use_b3)
    nc = _compiled[key]

    f8 = ml_dtypes.float8_e4m3
    # x: [B, T, D] -> fp8 [T, DP, B], pad row D=2752 carries bias (x=1),
    # then permute so each (t, k-pair, half) tile is contiguous:
    # rows (j k p) -> [core, T, j, half, p, k, hb]
    X = np.zeros((T, DP, B), dtype=f8)
    X[:, :D, :] = dvs.astype(f8).transpose(1, 2, 0)
    X[:, D, :] = f8(1.0)
    Xh = np.ascontiguousarray(
        X.reshape(T, KP, 2, 128, NCORES, 2, HB).transpose(4, 0, 1, 5, 3, 2, 6))

    w1p = np.zeros((DP, H), dtype=f8)
    w1p[:D, :] = (W1.T * WS).astype(f8)
    w1p[D, :] = (b1 * WS).astype(f8)
    w2p = np.ascontiguousarray((W2.T * WS).astype(f8))
    w3p = np.zeros((H, OP), dtype=f8)
    w3p[:, :O] = (W3.T * WS).astype(f8)
    b2p = (b2 * WS).astype(f8).reshape(1, H)
    b3p = np.zeros((1, OP), dtype=f8)
    b3p[0, :O] = (b3 * WS).astype(f8)

    in_maps = []
    for c in range(NCORES):
        in_maps.append({"x": Xh[c], "w1": w1p, "w2": w2p, "w3": w3p,
                        "b2": b2p, "b3": b3p})

    trace = bool(os.environ.get("SNN_TRACE"))
    last_results = run_bass_kernel_spmd(nc, in_maps, core_ids=list(range(NCORES)),
                                        trace=trace)
    out = np.empty((B, O), dtype=np.float32)
    for c in range(NCORES):
        out[c * BC:(c + 1) * BC, :] = last_results.results[c]["out"].T
    return out


# revision 9
# speedup vs baseline: 4.3860x; 4.3860x over previous
"""Trainium2 Bass kernel for NeuroVPR Vanilla SNN (3-layer LIF, T=3).

Data-parallel over batch: B=16384 -> 2048 per core x 8 cores.

Math (per timestep, per layer): v = (v_prev + h)/2; s = (v>=1); v *= (1-s).
All operands are fp8(e4m3) with weights pre-scaled by 16 (keeps N(0,1/D)
weight entries out of e4m3's subnormal range). Tracking W = 32*v:
    W_t = 0.5*M_{t-1} + H_t     (H = 16*h from scaled weights)
    s_t = (W_t >= 32)
    M_t = W_t * (W_t < 32)
Offline check vs the fp32 recurrence: ~10k/4.2M layer-1 spike flips, layer-2
membrane peaks at 0.59 vs threshold 1.0 (zero layer-2 spikes), so the output
spike pattern is unchanged.

Matmuls use fp8 MatmulPerfMode.DoubleRow (2 k-tiles of 128 per instruction,
2x PE rate): lhsT [128, 2, M] / rhs [128, 2, N] -> psum [M, N], N=512.
Layout: h.T = W @ x.T with d (contraction) on partitions. Host pre-permutes
dvs to [T, 11, half, 128, 2, 1024] fp8 so each k-pair half-batch tile is one
fully contiguous 256KB DMA (2KB per partition). D padded 2752->2816 (22*128);
pad row 2752 carries the L1 bias with x=1 there.

Schedule (keeps TensorE dense): per timestep, L1 runs as two half-batch
passes of 4 PSUM banks each, with the previous timestep's L2+L3 emitted
between the passes. Spike compares run on GpSimd; membrane updates on
VectorE; x DMAs alternate between the sync and scalar queues.
"""
import os
import numpy as np

B, T, D = 16384, 3, 2752
DP = 2816         # D padded to 22*128 (pad row 2752 = bias row)
H, O = 256, 100
OP = 112          # O padded to mult of 16 (DoubleRow ldweights step%16==0)
NCORES = 8
BC = B // NCORES  # 2048
NB = 512          # psum block along batch
KT = DP // 128    # 22 contraction tiles for L1
KP = KT // 2      # 11 DoubleRow k-pairs
HB = BC // 2      # half-batch per L1 pass (1024)
WS = 16.0         # weight pre-scale (power of 2)
TH = 2.0 * WS     # spike threshold in scaled-w units

_compiled = {}
last_results = None  # BassKernelResults of the most recent run (for profiling)


def _build(use_b2, use_b3):
    from contextlib import ExitStack
    import concourse.bass as bass
    import concourse.mybir as mybir
    import concourse.tile as tile
    from concourse import bacc

    f8, f32 = mybir.dt.float8e4, mybir.dt.float32
    A = mybir.AluOpType
    DR = mybir.MatmulPerfMode.DoubleRow

    nc = bacc.Bacc("TRN2", target_bir_lowering=False, debug=False)
    x = nc.dram_tensor("x", [T, KP, 2, 128, 2 * HB], f8, kind="ExternalInput").ap()
    w1 = nc.dram_tensor("w1", [DP, H], f8, kind="ExternalInput").ap()
    w2 = nc.dram_tensor("w2", [H, H], f8, kind="ExternalInput").ap()
    w3 = nc.dram_tensor("w3", [H, OP], f8, kind="ExternalInput").ap()
    b2 = nc.dram_tensor("b2", [1, H], f8, kind="ExternalInput").ap()
    b3 = nc.dram_tensor("b3", [1, OP], f8, kind="ExternalInput").ap()
    out = nc.dram_tensor("out", [O, BC], f32, kind="ExternalOutput").ap()

    with tile.TileContext(nc) as tc, ExitStack() as ctx:
        wp = ctx.enter_context(tc.tile_pool(name="wp", bufs=1))
        xp = ctx.enter_context(tc.tile_pool(name="xp", bufs=12))
        pp1 = ctx.enter_context(tc.tile_pool(name="pp1", bufs=6, space="PSUM"))
        pp23 = ctx.enter_context(tc.tile_pool(name="pp23", bufs=2, space="PSUM"))
        sp = ctx.enter_context(tc.tile_pool(name="sp", bufs=1))
        tp = ctx.enter_context(tc.tile_pool(name="tp", bufs=6))

        # resident weights, [d_part, (k h)] layout; pair j = dims k=2j,2j+1
        w1t = wp.tile([128, KT * H], f8)
        w1r = w1.rearrange("(k p) h -> p k h", p=128)
        w1o = w1t[:, :].rearrange("p (k h) -> p k h", k=KT)
        nc.sync.dma_start(out=w1o[:, 0:1, :], in_=w1r[:, 0:1, :])
        for c0, c1 in ((1, 7), (7, 14), (14, 22)):
            nc.scalar.dma_start(out=w1o[:, c0:c1, :], in_=w1r[:, c0:c1, :])
        w2t = wp.tile([128, 2 * H], f8)
        w2o = w2t[:, :].rearrange("p (k h) -> p k h", k=2)
        nc.gpsimd.dma_start(out=w2o, in_=w2.rearrange("(k p) h -> p k h", p=128))
        w3t = wp.tile([128, 2 * OP], f8)
        w3o = w3t[:, :].rearrange("p (k h) -> p k h", k=2)
        nc.gpsimd.dma_start(out=w3o, in_=w3.rearrange("(k p) h -> p k h", p=128))
        b2t = wp.tile([1, H], f8)
        nc.gpsimd.dma_start(out=b2t[:, :], in_=b2[:, :])
        b3t = wp.tile([1, OP], f8)
        nc.gpsimd.dma_start(out=b3t[:, :], in_=b3[:, :])
        ones = wp.tile([1, NB], f8)
        nc.gpsimd.memset(ones[:, :], 1.0)

        # persistent state (M = 32*v_after_reset, zero-init) and fp8 spikes
        m1 = [sp.tile([128, BC], f32, tag=f"m1_{h}", name=f"m1_{h}") for h in range(2)]
        m2 = [sp.tile([128, BC], f32, tag=f"m2_{h}", name=f"m2_{h}") for h in range(2)]
        m3 = sp.tile([128, BC], f32, tag="m3")
        s1 = sp.tile([128, 2 * BC], f8, tag="s1", name="s1")
        s2 = sp.tile([128, 2 * BC], f8, tag="s2", name="s2")
        s1v = s1[:, :].rearrange("p (k b) -> p k b", k=2)
        s2v = s2[:, :].rearrange("p (k b) -> p k b", k=2)
        outsb = sp.tile([128, BC], f32, tag="outsb")
        for mt in (*m1, *m2, m3):
            nc.vector.memset(mt[:, :], 0.0)

        def lif_w(psum, m_ap):
            """W = M/2 + H. Reads+releases the psum bank; returns w tile."""
            P = psum.shape[0]
            w = tp.tile([128, NB], f32, tag="w", name="w")[:P, :]
            nc.vector.scalar_tensor_tensor(w, m_ap, 0.5, psum, A.mult, A.add)
            return w

        def lif_s(w, s_ap):
            nc.vector.tensor_scalar(s_ap, w, TH, None, A.is_ge)

        def lif_m(w, m_ap):
            nc.vector.scalar_tensor_tensor(m_ap, w, TH, w, A.is_lt, A.mult)

        def l2_group(t, h, b, pool, tag):
            ps2 = pool.tile([128, NB], f32, tag=tag, name=f"ps2_{t}_{h}_{b}")
            first = True
            if use_b2:
                nc.tensor.matmul(ps2[:, :], b2t[0:1, h * 128:(h + 1) * 128],
                                 ones[0:1, :], start=True, stop=False)
                first = False
            nc.tensor.matmul(ps2[:, :], w2o[:, 0:2, h * 128:(h + 1) * 128],
                             s1v[:, :, b * NB:(b + 1) * NB],
                             start=first, stop=True, perf_mode=DR)
            return ps2

        def l3_group(t, b, pool, tag):
            ps3 = pool.tile([128, NB], f32, tag=tag, name=f"ps3_{t}_{b}")
            first = True
            if use_b3:
                nc.tensor.matmul(ps3[:OP, :], b3t[0:1, :], ones[0:1, :],
                                 start=True, stop=False)
                first = False
            nc.tensor.matmul(ps3[:OP, :], w3o[:, 0:2, 0:OP],
                             s2v[:, :, b * NB:(b + 1) * NB],
                             start=first, stop=True, perf_mode=DR)
            return ps3

        def l2_all(t, pool, tag):
            """Layer-2 matmuls + LIF for timestep t (all batch blocks)."""
            last = (t == T - 1)
            for b in range(4):
                bs = slice(b * NB, (b + 1) * NB)
                for h in range(2):
                    ps2 = l2_group(t, h, b, pool, tag)
                    w = lif_w(ps2[:, :], m2[h][:, bs])
                    lif_s(w, s2v[:, h, bs])
                    if not last:
                        lif_m(w, m2[h][:, bs])

        def l3_all(t, pool, tag):
            """Layer-3 matmuls + LIF for timestep t (all batch blocks)."""
            last = (t == T - 1)
            for b in range(4):
                bs = slice(b * NB, (b + 1) * NB)
                ps3 = l3_group(t, b, pool, tag)
                w3_ = lif_w(ps3[:O, :], m3[:O, bs])
                if not last:
                    # s3 is only consumed at the last timestep; the reset
                    # uses (w<TH) directly, so skip the spike compare here.
                    lif_m(w3_, m3[:O, bs])
                else:
                    lif_s(w3_, outsb[:O, bs])
                    nc.sync.dma_start(out=out[:, bs], in_=outsb[:O, bs])

        def l1_pass(t, half):
            """One half-batch L1 pass: 4 psum groups (2h x 2b), k-pairs inner."""
            boff = half * HB
            ps1 = [[pp1.tile([128, NB], f32, tag="ps1", name=f"ps1_{t}_{half}_{h}_{b}")
                    for b in range(2)] for h in range(2)]
            for j in range(KP):
                xt = xp.tile([128, 2 * HB], f8, tag="x", name="xt")
                q = nc.sync if (j % 2 == 0) else nc.scalar
                q.dma_start(out=xt[:, :], in_=x[t, j, half, :, :])
                xv = xt[:, :].rearrange("p (k b) -> p k b", k=2)
                for h in range(2):
                    for b in range(2):
                        nc.tensor.matmul(
                            ps1[h][b][:, :],
                            w1o[:, 2 * j:2 * j + 2, h * 128:(h + 1) * 128],
                            xv[:, :, b * NB:(b + 1) * NB],
                            start=(j == 0), stop=(j == KP - 1), perf_mode=DR)
            # release all 4 banks first (w-ops), then spikes, then membranes
            ws = {}
            for h in range(2):
                for b in range(2):
                    bs = slice(boff + b * NB, boff + (b + 1) * NB)
                    ws[h, b] = lif_w(ps1[h][b][:, :], m1[h][:, bs])
            for h in range(2):
                for b in range(2):
                    bs = slice(boff + b * NB, boff + (b + 1) * NB)
                    lif_s(ws[h, b], s1v[:, h, bs])
            if t != T - 1:
                for h in range(2):
                    for b in range(2):
                        bs = slice(boff + b * NB, boff + (b + 1) * NB)
                        lif_m(ws[h, b], m1[h][:, bs])

        for t in range(T):
            l1_pass(t, 0)
            if t > 0:
                l2_all(t - 1, pp23, "ps23")
            if t == T - 1:
                l3_all(t - 1, pp23, "ps23")
                for b in (0, 1):
                    bs = slice(b * NB, (b + 1) * NB)
                    for h in range(2):
                        ps2 = l2_group(t, h, b, pp23, "ps23")
                        w = lif_w(ps2[:, :], m2[h][:, bs])
                        lif_s(w, s2v[:, h, bs])
            l1_pass(t, 1)
            if 0 < t < T - 1:
                l3_all(t - 1, pp23, "ps23")
        # tail: l2(T-1, b23) and l3(T-1) pipelined per b-block
        t_ = T - 1
        for b in (2, 3):
            bs = slice(b * NB, (b + 1) * NB)
            for h in range(2):
                ps2 = l2_group(t_, h, b, pp1, "ps1")
                w = lif_w(ps2[:, :], m2[h][:, bs])
                lif_s(w, s2v[:, h, bs])
            bp = b - 2
            bs = slice(bp * NB, (bp + 1) * NB)
            ps3 = l3_group(t_, bp, pp23, "ps23")
            w3_ = lif_w(ps3[:O, :], m3[:O, bs])
            lif_s(w3_, outsb[:O, bs])
            nc.sync.dma_start(out=out[:, bs], in_=outsb[:O, bs])
        for bp in (2, 3):
            bs = slice(bp * NB, (bp + 1) * NB)
            ps3 = l3_group(t_, bp, pp23, "ps23")
            w3_ = lif_w(ps3[:O, :], m3[:O, bs])
            lif_s(w3_, outsb[:O, bs])
            nc.sync.dma_start(out=out[:, bs], in_=outsb[:O, bs])

    nc.compile()
    return nc


def kernel(dvs, W1, b1, W2, b2, W3, b3):
    global last_results
    import ml_dtypes
    from concourse.bass_utils import run_bass_kernel_spmd

    use_b2 = bool(np.any(b2))
    use_b3 = bool(np.any(b3))
    key = (use_b2, use_b3)
    if key not in _compiled:
        _compiled[key] = _build(use_b2, use_b3)
    nc = _compiled[key]

    f8 = ml_dtypes.float8_e4m3
    # x: [B, T, D] -> fp8 [T, DP, B], pad row D=2752 carries bias (x=1),
    # then permute so each (t, k-pair, half) tile is contiguous:
    # rows (j k p) -> [core, T, j, half, p, k, hb]
    X = np.zeros((T, DP, B), dtype=f8)
    X[:, :D, :] = dvs.astype(f8).transpose(1, 2, 0)
    X[:, D, :] = f8(1.0)
    Xh = np.ascontiguousarray(
        X.reshape(T, KP, 2, 128, NCORES, 2, HB).transpose(4, 0, 1, 5, 3, 2, 6))

    w1p = np.zeros((DP, H), dtype=f8)
    w1p[:D, :] = (W1.T * WS).astype(f8)
    w1p[D, :] = (b1 * WS).astype(f8)
    w2p = np.ascontiguousarray((W2.T * WS).astype(f8))
    w3p = np.zeros((H, OP), dtype=f8)
    w3p[:, :O] = (W3.T * WS).astype(f8)
    b2p = (b2 * WS).astype(f8).reshape(1, H)
    b3p = np.zeros((1, OP), dtype=f8)
    b3p[0, :O] = (b3 * WS).astype(f8)

    in_maps = []
    for c in range(NCORES):
        in_maps.append({"x": Xh[c], "w1": w1p, "w2": w2p, "w3": w3p,
                        "b2": b2p, "b3": b3p})

    trace = bool(os.environ.get("SNN_TRACE"))
    last_results = run_bass_kernel_spmd(nc, in_maps, core_ids=list(range(NCORES)),
                                        trace=trace)
    out = np.empty((B, O), dtype=np.float32)
    for c in range(NCORES):
        out[c * BC:(c + 1) * BC, :] = last_results.results[c]["out"].T
    return out
